# revision 5
# baseline (speedup 1.0000x reference)
"""Trainium2 Bass kernel for the relational GCN layer (gnn_message_passing).

Math (from the reference):
    out[n, e, i] = sum_k sum_m sum_d adj[n, m, k] * x[m, d, (i-k)%4] * W[d, e, k]

Factored for the PE (contraction dim must sit on SBUF partitions):
    X4[m, f]   = x.reshape(4096, 128)            with f = d*4 + j
    G_k[f, n]  = sum_m X4[m, f] * adj[n, m, k]   (the big 256 MB contraction)
    outT[c, n] = sum_k sum_f Wbig[f, k, c] * G_k[f, n]   with c = e*4 + i
    Wbig[d*4+j, k, e*4+i] = W[d, e, k] if j == (i-k)%4 else 0

Precision: fp32 matmuls on trn2 lower to 2 HW passes x 2 cycles/col (4x the
2-byte rate).  Instead adj and x are split into fp16 hi+lo pairs and G is
accumulated as 3 half-rate products (ah*xh + al*xh + ah*xl) in fp32 PSUM —
22 effective mantissa bits, measured ~5e-7 rel err (fp32-level), at 3
cycles/col of PE time.  The dropped al*xl term is ~2^-22 relative.

Sharding: 1D over the node (row) dim of adj/out — core c owns rows
[c*512, (c+1)*512).  x and the (tiny) weight are replicated.  adj is
pre-packed on the host into hi/lo fp16 tiles laid out exactly as the PE
streams them ([m-partition, n-free], contiguous per partition per DMA), so
the kernel runs at the HBM roofline with zero on-chip transposes.
"""

import numpy as np

N_CORES = 8
NODES = 4096
N_PER_CORE = NODES // N_CORES          # 512
F = 128                                # d*4+j
C = 128                                # e*4+i
MB = 32                                # m-chunks of 128 (4096 / 128)
GROUPS = 8                             # DMA groups of 4 m-chunks (4 MB each)
MB_PER_GROUP = MB // GROUPS            # 4
R = 4
GROUP_COLS = MB_PER_GROUP * R * 2 * N_PER_CORE   # (a, k, hl, nn) = 16384

_PATCHED = False
_PROG = None


def _patch_tile_drain():
    """This container's walrus build rejects >2 sync waits on one Drain;
    split the Tile end-of-context drain into one single-wait drain per proc
    (semantically identical: the SP engine observes each clock lane in
    sequence before the barrier)."""
    global _PATCHED
    if _PATCHED:
        return
    from concourse.tile import TileContext
    from concourse.vector_clock import ScopedClock, VectorClock
    from concourse.tile_scheduler import N_PROCS

    def _split_drain_and_barrier(self, tick_clock, wait_clock):
        g = tick_clock.global_clock
        for p in range(N_PROCS):
            if g[p] > 0:
                d = self.nc.sync.drain()
                pc = VectorClock([g[q] if q == p else 0 for q in range(N_PROCS)])
                wait_clock.add_sem_waits(d.ins, ScopedClock({None: pc}))
        self.nc.all_engine_barrier()
        assert self.sems is not None
        popped = self.nc._tile_sem_poison_stack.pop()
        assert popped is self._sem_poison
        self.nc.clear_and_free_semaphores(list(self.sems.allocated().values()))
        self.nc.all_engine_barrier()

    TileContext._drain_and_barrier = _split_drain_and_barrier
    _PATCHED = True


def _split_sync_waits(bir_bytes, max_waits=1):
    """This container's walrus build rejects instructions carrying more than
    ~2 sync waits.  Hoist all but one wait of any instruction onto standalone
    EventSemaphore instructions on the same engine immediately before it —
    the engine then observes the semaphores sequentially, which is
    semantically identical."""
    import json
    j = json.loads(bir_bytes)
    n_new = 0
    for f in j.get("functions", []):
        for bb in f.get("blocks", []):
            out_insts = []
            for inst in bb.get("instructions", []):
                waits = (inst.get("sync_info") or {}).get("on_wait") or []
                if len(waits) > max_waits:
                    keep = waits[-max_waits:]
                    for w in waits[:-max_waits]:
                        n_new += 1
                        ev = {
                            "engine": inst["engine"],
                            "ins": [],
                            "name": f"{inst['name']}_wsplit{n_new}",
                            "opcode": "EventSemaphore",
                            "outs": [],
                            "sync_info": {"on_update": [], "on_wait": [w]},
                        }
                        if "debug" in inst:
                            ev["debug"] = inst["debug"]
                        out_insts.append(ev)
                    inst["sync_info"]["on_wait"] = keep
                out_insts.append(inst)
            bb["instructions"] = out_insts
    return json.dumps(j).encode()


def _build_program():
    global _PROG
    if _PROG is not None:
        return _PROG
    _patch_tile_drain()
    import concourse.bass as bass
    import concourse.mybir as mybir
    from concourse.tile import TileContext

    f32 = mybir.dt.float32
    f16 = mybir.dt.float16
    nc = bass.Bass()
    # adjt[g, mp, (a, k, hl, nn)]: hi/lo fp16 of adj[n0+nn, (4g+a)*128+mp, k]
    adjt = nc.dram_tensor("adjt", [GROUPS, 128, GROUP_COLS], f16,
                          kind="ExternalInput")
    # x4h/x4l[mp, mb, f]: hi/lo fp16 of x.reshape(4096, 128)[mb*128+mp, f]
    x4h = nc.dram_tensor("x4h", [128, MB, F], f16, kind="ExternalInput")
    x4l = nc.dram_tensor("x4l", [128, MB, F], f16, kind="ExternalInput")
    # wbigt[f, k, c] fp32
    wbigt = nc.dram_tensor("wbigt", [F, R, C], f32, kind="ExternalInput")
    outt = nc.dram_tensor("outt", [C, N_PER_CORE], f32, kind="ExternalOutput")

    with TileContext(nc) as tc:
        with (
            tc.tile_pool(name="const", bufs=1) as cpool,
            tc.tile_pool(name="adj", bufs=3) as apool,
            tc.tile_pool(name="gout", bufs=1) as gpool,
            tc.tile_pool(name="psum", bufs=1, space="PSUM") as ppool,
        ):
            # small inputs ride the Activation HWDGE ring so the first adj
            # group (on the SP ring) isn't queued behind them
            xhsb = cpool.tile([128, MB, F], f16)
            nc.scalar.dma_start(out=xhsb[:, :, :], in_=x4h[:, :, :])
            xlsb = cpool.tile([128, MB, F], f16)
            nc.scalar.dma_start(out=xlsb[:, :, :], in_=x4l[:, :, :])
            wsb = cpool.tile([F, R, C], f32)
            nc.scalar.dma_start(out=wsb[:, :, :], in_=wbigt[:, :, :])

            gps = [ppool.tile([F, N_PER_CORE], f32, tag=f"g{k}", name=f"gps{k}")
                   for k in range(R)]

            def rhs(adjsb, a, k, hl):
                off = ((a * R + k) * 2 + hl) * N_PER_CORE
                return adjsb[:, off:off + N_PER_CORE]

            for g in range(GROUPS):
                adjsb = apool.tile([128, GROUP_COLS], f16)
                nc.sync.dma_start(out=adjsb[:, :], in_=adjt[g, :, :])
                for a in range(MB_PER_GROUP):
                    mb = g * MB_PER_GROUP + a
                    # stationary xh: ah then al streams; stationary xl: ah
                    for k in range(R):
                        nc.tensor.matmul(gps[k][:, :], lhsT=xhsb[:, mb, :],
                                         rhs=rhs(adjsb, a, k, 0),
                                         start=(mb == 0), stop=False)
                    for k in range(R):
                        nc.tensor.matmul(gps[k][:, :], lhsT=xhsb[:, mb, :],
                                         rhs=rhs(adjsb, a, k, 1),
                                         start=False, stop=False)
                    for k in range(R):
                        nc.tensor.matmul(gps[k][:, :], lhsT=xlsb[:, mb, :],
                                         rhs=rhs(adjsb, a, k, 0),
                                         start=False, stop=(mb == MB - 1))

            gsb = gpool.tile([F, R, N_PER_CORE], f32)
            for k in range(R):
                nc.vector.tensor_copy(gsb[:, k, :], gps[k][:, :])

            ops = ppool.tile([C, N_PER_CORE], f32, tag="out")
            for k in range(R):
                nc.tensor.matmul(ops[:, :], lhsT=wsb[:, k, :], rhs=gsb[:, k, :],
                                 start=(k == 0), stop=(k == R - 1))
            osb = gpool.tile([C, N_PER_CORE], f32, tag="osb")
            nc.vector.tensor_copy(osb[:, :], ops[:, :])
            nc.sync.dma_start(out=outt[:, :], in_=osb[:, :])

    _orig_to_json = nc.to_json_bytes
    nc.to_json_bytes = lambda: _split_sync_waits(_orig_to_json())

    _PROG = nc
    return nc


def _pack_adj(adj):
    """adj [4096, 4096, 4] f32 -> per-core [GROUPS, 128, GROUP_COLS] fp16
    hi/lo with adjt[c][g, mp, (a, k, hl, nn)] = hl-part of
    adj[c*512+nn, (4g+a)*128+mp, k]."""
    A = adj.reshape(N_CORES, N_PER_CORE, GROUPS, MB_PER_GROUP, 128, R)
    At = np.ascontiguousarray(A.transpose(0, 2, 4, 3, 5, 1))  # [c,g,mp,a,k,nn]
    hi = At.astype(np.float16)
    lo = (At - hi.astype(np.float32)).astype(np.float16)
    out = np.empty((N_CORES, GROUPS, 128, MB_PER_GROUP, R, 2, N_PER_CORE),
                   np.float16)
    out[..., 0, :] = hi
    out[..., 1, :] = lo
    return out.reshape(N_CORES, GROUPS, 128, GROUP_COLS)


def _prepare_in_maps(x, adj, weight):
    x = np.ascontiguousarray(np.asarray(x), dtype=np.float32)
    adj = np.ascontiguousarray(np.asarray(adj), dtype=np.float32)
    weight = np.ascontiguousarray(np.asarray(weight), dtype=np.float32)

    x4 = np.ascontiguousarray(
        x.reshape(MB, 128, F).transpose(1, 0, 2))          # [mp, mb, f]
    x4h = x4.astype(np.float16)
    x4l = (x4 - x4h.astype(np.float32)).astype(np.float16)
    wbigt = np.zeros((F, R, C), np.float32)                # [f, k, c]
    for k in range(R):
        for i in range(R):
            j = (i - k) % R
            wbigt[j::R, k, i::R] = weight[:, :, k]
    adjt = _pack_adj(adj)
    return [{"adjt": adjt[c], "x4h": x4h, "x4l": x4l, "wbigt": wbigt}
            for c in range(N_CORES)]


def _assemble_out(results):
    outt = np.stack([r["outt"] for r in results])          # [8, 128, 512]
    out = outt.reshape(N_CORES, 32, R, N_PER_CORE)         # [c, e, i, nn]
    out = out.transpose(0, 3, 1, 2).reshape(NODES, 32, R)  # [n, e, i]
    return np.ascontiguousarray(out)


def kernel(x, adj, weight):
    nc = _build_program()
    in_maps = _prepare_in_maps(x, adj, weight)
    from concourse.bass_utils import run_bass_kernel_spmd
    res = run_bass_kernel_spmd(nc, in_maps, core_ids=list(range(N_CORES)))
    return _assemble_out(res.results)


# revision 7
# speedup vs baseline: 1.1416x; 1.1416x over previous
"""Trainium2 Bass kernel for the relational GCN layer (gnn_message_passing).

Math (from the reference):
    out[n, e, i] = sum_k sum_m sum_d adj[n, m, k] * x[m, d, (i-k)%4] * W[d, e, k]

Factored for the PE (contraction dim must sit on SBUF partitions):
    X4[m, f]   = x.reshape(4096, 128)            with f = d*4 + j
    G_k[f, n]  = sum_m X4[m, f] * adj[n, m, k]   (the big 256 MB contraction)
    outT[c, n] = sum_k sum_f Wbig[f, k, c] * G_k[f, n]   with c = e*4 + i
    Wbig[d*4+j, k, e*4+i] = W[d, e, k] if j == (i-k)%4 else 0

Precision: fp32 matmuls on trn2 lower to 2 HW passes x 2 cycles/col (4x the
2-byte rate).  Instead adj and x are split into fp16 hi+lo pairs and G is
accumulated as 3 half-rate products (ah*xh + al*xh + ah*xl) in fp32 PSUM —
22 effective mantissa bits, measured ~5e-7 rel err (fp32-level), at 3
cycles/col of PE time.  The dropped al*xl term is ~2^-22 relative.

Sharding: 1D over the node (row) dim of adj/out — core c owns rows
[c*512, (c+1)*512).  x and the (tiny) weight are replicated.  adj is
pre-packed on the host into hi/lo fp16 tiles laid out exactly as the PE
streams them ([m-partition, n-free], contiguous per partition per DMA), so
the kernel runs at the HBM roofline with zero on-chip transposes.
"""

import numpy as np

N_CORES = 8
NODES = 4096
N_PER_CORE = NODES // N_CORES          # 512
F = 128                                # d*4+j
C = 128                                # e*4+i
MB = 32                                # m-chunks of 128 (4096 / 128)
GROUPS = 8                             # DMA groups of 4 m-chunks (4 MB each)
MB_PER_GROUP = MB // GROUPS            # 4
R = 4
GROUP_COLS = MB_PER_GROUP * R * 2 * N_PER_CORE   # (a, k, hl, nn) = 16384

_PATCHED = False
_PROG = None


def _patch_tile_drain():
    """This container's walrus build rejects >2 sync waits on one Drain;
    split the Tile end-of-context drain into one single-wait drain per proc
    (semantically identical: the SP engine observes each clock lane in
    sequence before the barrier)."""
    global _PATCHED
    if _PATCHED:
        return
    from concourse.tile import TileContext
    from concourse.vector_clock import ScopedClock, VectorClock
    from concourse.tile_scheduler import N_PROCS

    def _split_drain_and_barrier(self, tick_clock, wait_clock):
        g = tick_clock.global_clock
        for p in range(N_PROCS):
            if g[p] > 0:
                d = self.nc.sync.drain()
                pc = VectorClock([g[q] if q == p else 0 for q in range(N_PROCS)])
                wait_clock.add_sem_waits(d.ins, ScopedClock({None: pc}))
        self.nc.all_engine_barrier()
        assert self.sems is not None
        popped = self.nc._tile_sem_poison_stack.pop()
        assert popped is self._sem_poison
        self.nc.clear_and_free_semaphores(list(self.sems.allocated().values()))
        self.nc.all_engine_barrier()

    TileContext._drain_and_barrier = _split_drain_and_barrier
    _PATCHED = True


def _split_sync_waits(bir_bytes, max_waits=1):
    """This container's walrus build rejects instructions carrying more than
    ~2 sync waits.  Hoist all but one wait of any instruction onto standalone
    EventSemaphore instructions on the same engine immediately before it —
    the engine then observes the semaphores sequentially, which is
    semantically identical."""
    import json
    j = json.loads(bir_bytes)
    n_new = 0
    for f in j.get("functions", []):
        for bb in f.get("blocks", []):
            out_insts = []
            for inst in bb.get("instructions", []):
                waits = (inst.get("sync_info") or {}).get("on_wait") or []
                if len(waits) > max_waits:
                    keep = waits[-max_waits:]
                    for w in waits[:-max_waits]:
                        n_new += 1
                        ev = {
                            "engine": inst["engine"],
                            "ins": [],
                            "name": f"{inst['name']}_wsplit{n_new}",
                            "opcode": "EventSemaphore",
                            "outs": [],
                            "sync_info": {"on_update": [], "on_wait": [w]},
                        }
                        if "debug" in inst:
                            ev["debug"] = inst["debug"]
                        out_insts.append(ev)
                    inst["sync_info"]["on_wait"] = keep
                out_insts.append(inst)
            bb["instructions"] = out_insts
    return json.dumps(j).encode()


def _build_program():
    global _PROG
    if _PROG is not None:
        return _PROG
    _patch_tile_drain()
    import concourse.bass as bass
    import concourse.mybir as mybir
    from concourse.tile import TileContext

    f32 = mybir.dt.float32
    f16 = mybir.dt.float16
    nc = bass.Bass()
    # adjt[g, mp, (a, k, hl, nn)]: hi/lo fp16 of adj[n0+nn, (4g+a)*128+mp, k]
    adjt = nc.dram_tensor("adjt", [GROUPS, 128, GROUP_COLS], f16,
                          kind="ExternalInput")
    # x4hl[mp, hl, mb, f]: hi/lo fp16 of x.reshape(4096, 128)[mb*128+mp, f];
    # one tensor so the DMA moves 16 KB contiguous per partition
    x4hl = nc.dram_tensor("x4hl", [128, 2, MB, F], f16, kind="ExternalInput")
    # wbigt[f, k, c] fp32
    wbigt = nc.dram_tensor("wbigt", [F, R, C], f32, kind="ExternalInput")
    outt = nc.dram_tensor("outt", [C, N_PER_CORE], f32, kind="ExternalOutput")

    with TileContext(nc) as tc:
        with (
            tc.tile_pool(name="const", bufs=1) as cpool,
            tc.tile_pool(name="adj", bufs=3) as apool,
            tc.tile_pool(name="gout", bufs=1) as gpool,
            tc.tile_pool(name="psum", bufs=1, space="PSUM") as ppool,
        ):
            # HAM warmup: ~4 us of dummy matmuls on a zeroed tile so the PE
            # clock-gate is at 8/8 by the time real data arrives (PE is
            # otherwise idle while the first DMA group lands).
            warm = cpool.tile([128, F], f16)
            nc.vector.memset(warm[:, :], 0.0)
            wps = ppool.tile([128, 64], f32, tag="warm")
            for _ in range(48):
                nc.tensor.matmul(wps[:, :], lhsT=warm[:, :], rhs=warm[:, :64],
                                 start=True, stop=True)

            # small inputs ride the Activation HWDGE ring so the first adj
            # group (on the SP ring) isn't queued behind them
            xsb = cpool.tile([128, 2, MB, F], f16)
            nc.scalar.dma_start(out=xsb[:, :, :, :], in_=x4hl[:, :, :, :])
            wsb = cpool.tile([F, R, C], f32)
            nc.scalar.dma_start(out=wsb[:, :, :], in_=wbigt[:, :, :])

            gps = [ppool.tile([F, N_PER_CORE], f32, tag=f"g{k}", name=f"gps{k}")
                   for k in range(R)]

            def rhs(adjsb, a, k, hl):
                off = ((a * R + k) * 2 + hl) * N_PER_CORE
                return adjsb[:, off:off + N_PER_CORE]

            for g in range(GROUPS):
                adjsb = apool.tile([128, GROUP_COLS], f16)
                # alternate the two HWDGE rings for throughput
                dma_eng = nc.sync if g % 2 == 0 else nc.scalar
                dma_eng.dma_start(out=adjsb[:, :], in_=adjt[g, :, :])
                for a in range(MB_PER_GROUP):
                    mb = g * MB_PER_GROUP + a
                    # stationary xh: ah then al streams; stationary xl: ah
                    for k in range(R):
                        nc.tensor.matmul(gps[k][:, :], lhsT=xsb[:, 0, mb, :],
                                         rhs=rhs(adjsb, a, k, 0),
                                         start=(mb == 0), stop=False)
                    for k in range(R):
                        nc.tensor.matmul(gps[k][:, :], lhsT=xsb[:, 0, mb, :],
                                         rhs=rhs(adjsb, a, k, 1),
                                         start=False, stop=False)
                    for k in range(R):
                        nc.tensor.matmul(gps[k][:, :], lhsT=xsb[:, 1, mb, :],
                                         rhs=rhs(adjsb, a, k, 0),
                                         start=False, stop=(mb == MB - 1))

            gsb = gpool.tile([F, R, N_PER_CORE], f32)
            for k in range(R):
                nc.vector.tensor_copy(gsb[:, k, :], gps[k][:, :])

            ops = ppool.tile([C, N_PER_CORE], f32, tag="out")
            for k in range(R):
                nc.tensor.matmul(ops[:, :], lhsT=wsb[:, k, :], rhs=gsb[:, k, :],
                                 start=(k == 0), stop=(k == R - 1))
            osb = gpool.tile([C, N_PER_CORE], f32, tag="osb")
            nc.vector.tensor_copy(osb[:, :], ops[:, :])
            nc.sync.dma_start(out=outt[:, :], in_=osb[:, :])

    _orig_to_json = nc.to_json_bytes
    nc.to_json_bytes = lambda: _split_sync_waits(_orig_to_json())

    _PROG = nc
    return nc


def _pack_adj(adj):
    """adj [4096, 4096, 4] f32 -> per-core [GROUPS, 128, GROUP_COLS] fp16
    hi/lo with adjt[c][g, mp, (a, k, hl, nn)] = hl-part of
    adj[c*512+nn, (4g+a)*128+mp, k]."""
    A = adj.reshape(N_CORES, N_PER_CORE, GROUPS, MB_PER_GROUP, 128, R)
    At = np.ascontiguousarray(A.transpose(0, 2, 4, 3, 5, 1))  # [c,g,mp,a,k,nn]
    hi = At.astype(np.float16)
    lo = (At - hi.astype(np.float32)).astype(np.float16)
    out = np.empty((N_CORES, GROUPS, 128, MB_PER_GROUP, R, 2, N_PER_CORE),
                   np.float16)
    out[..., 0, :] = hi
    out[..., 1, :] = lo
    return out.reshape(N_CORES, GROUPS, 128, GROUP_COLS)


def _prepare_in_maps(x, adj, weight):
    x = np.ascontiguousarray(np.asarray(x), dtype=np.float32)
    adj = np.ascontiguousarray(np.asarray(adj), dtype=np.float32)
    weight = np.ascontiguousarray(np.asarray(weight), dtype=np.float32)

    x4 = np.ascontiguousarray(
        x.reshape(MB, 128, F).transpose(1, 0, 2))          # [mp, mb, f]
    x4hl = np.empty((128, 2, MB, F), np.float16)
    x4hl[:, 0] = x4.astype(np.float16)
    x4hl[:, 1] = (x4 - x4hl[:, 0].astype(np.float32)).astype(np.float16)
    wbigt = np.zeros((F, R, C), np.float32)                # [f, k, c]
    for k in range(R):
        for i in range(R):
            j = (i - k) % R
            wbigt[j::R, k, i::R] = weight[:, :, k]
    adjt = _pack_adj(adj)
    return [{"adjt": adjt[c], "x4hl": x4hl, "wbigt": wbigt}
            for c in range(N_CORES)]


def _assemble_out(results):
    outt = np.stack([r["outt"] for r in results])          # [8, 128, 512]
    out = outt.reshape(N_CORES, 32, R, N_PER_CORE)         # [c, e, i, nn]
    out = out.transpose(0, 3, 1, 2).reshape(NODES, 32, R)  # [n, e, i]
    return np.ascontiguousarray(out)


def kernel(x, adj, weight):
    nc = _build_program()
    in_maps = _prepare_in_maps(x, adj, weight)
    from concourse.bass_utils import run_bass_kernel_spmd
    res = run_bass_kernel_spmd(nc, in_maps, core_ids=list(range(N_CORES)))
    return _assemble_out(res.results)
